# revision 2
# baseline (speedup 1.0000x reference)
"""MoE routing kernel for Trainium2 (Bass/Tile), 8-core data-parallel.

Problem: out = einsum('be,beo->bo', softmax(x@Wg+bg, axis=1),
                      einsum('bd,edo->beo', x, We) + be)
with B=8192, D=1024, O=1024, E=8 (all experts dense, softmax-weighted).

Strategy: shard the batch across 8 NeuronCores (1024 rows each). Each core:
  - computes gates = softmax(x@Wg + bg) on-chip (free-dim softmax),
  - transposes gates (PE transpose) to form gT for the bias term g@be,
  - for each expert: accumulates x@We[e] in PSUM (bf16 matmuls, fp32 acc),
  - combines with one fused DVE op per expert tile:
        acc = psum_e * g[:,e] + acc   (scalar_tensor_tensor)
  - acc is seeded with the bias term g@be (one K=8 matmul per tile).
Inputs are cast to bf16 host-side (x additionally pre-transposed to [D, Bs]
so it can serve as the stationary matmul operand directly).
"""
from contextlib import ExitStack

import numpy as np
import ml_dtypes

import concourse.tile as tile
import concourse.mybir as mybir
from concourse import bacc
from concourse.bass_utils import run_bass_kernel_spmd
from concourse.masks import make_identity

B, D, O, E = 8192, 1024, 1024, 8
NCORES = 8
BS = B // NCORES          # batch rows per core
P = 128                   # partition dim
NT = 512                  # matmul moving free-dim / PSUM bank width (fp32)
KC = D // P               # contraction chunks (8)
MC = BS // P              # batch-row chunks per core (8)
NCH = O // NT             # output column chunks (2)

F32 = mybir.dt.float32
BF16 = mybir.dt.bfloat16
MULT = mybir.AluOpType.mult
ADD = mybir.AluOpType.add


def _emit(nc, tc, xT, We, Wg, bg, be, out):
    ctx = ExitStack()
    with ctx:
        const = ctx.enter_context(tc.tile_pool(name="const", bufs=1))
        xp = ctx.enter_context(tc.tile_pool(name="xp", bufs=1))
        wp = ctx.enter_context(tc.tile_pool(name="wp", bufs=1))
        gp = ctx.enter_context(tc.tile_pool(name="gp", bufs=1))
        accp = ctx.enter_context(tc.tile_pool(name="accp", bufs=2))
        small = ctx.enter_context(tc.tile_pool(name="small", bufs=2))
        gps = ctx.enter_context(tc.tile_pool(name="gps", bufs=2, space="PSUM"))
        tps = ctx.enter_context(tc.tile_pool(name="tps", bufs=1, space="PSUM"))
        bps = ctx.enter_context(tc.tile_pool(name="bps", bufs=2, space="PSUM"))
        eps = ctx.enter_context(tc.tile_pool(name="eps", bufs=3, space="PSUM"))

        # ---- loads ----
        # DMA emission order = queue fill order: small gate constants first,
        # then xT (gate matmuls need every k-chunk), then expert-0 weights so
        # the expert stream can start, then the remaining experts.
        wg_sb = []
        for k in range(KC):
            t = const.tile([P, E], BF16, name=f"wg{k}", tag=f"wg{k}")
            nc.sync.dma_start(t[:], Wg[k * P:(k + 1) * P, :])
            wg_sb.append(t)

        bg_sb = const.tile([1, E], F32, name="bg_sb")
        nc.sync.dma_start(bg_sb[:], bg)
        be_sb = const.tile([E, O], BF16, name="be_sb")
        nc.sync.dma_start(be_sb[:], be)

        ones_sb = const.tile([1, P], F32, name="ones_sb")
        nc.vector.memset(ones_sb[:], 1.0)
        ident = const.tile([P, P], F32, name="ident")
        make_identity(nc, ident[:])

        xt_sb = []
        for k in range(KC):
            t = xp.tile([P, BS], BF16, name=f"xt{k}", tag=f"xt{k}")
            nc.sync.dma_start(t[:], xT[k * P:(k + 1) * P, :])
            xt_sb.append(t)

        we_sb = [[None] * KC for _ in range(E)]
        for e in range(E):
            for k in range(KC):
                t = wp.tile([P, O], BF16, name=f"we{e}_{k}", tag=f"we{e}_{k}")
                nc.sync.dma_start(t[:], We[e, k * P:(k + 1) * P, :])
                we_sb[e][k] = t

        # ---- gates: softmax(x @ Wg + bg) ----
        gates_sb = []
        gT_all = gp.tile([E, BS], BF16, name="gT_all")
        for m in range(MC):
            ms = slice(m * P, (m + 1) * P)
            pg = gps.tile([P, E], F32, name="pg", tag="pg")
            for k in range(KC):
                nc.tensor.matmul(pg[:], xt_sb[k][:, ms], wg_sb[k][:],
                                 start=(k == 0), stop=False)
            nc.tensor.matmul(pg[:], ones_sb[:], bg_sb[:], start=False, stop=True)

            rmax = small.tile([P, 1], F32, name="rmax", tag="rmax")
            nc.vector.tensor_reduce(rmax[:], pg[:], axis=mybir.AxisListType.X,
                                    op=mybir.AluOpType.max)
            nmax = small.tile([P, 1], F32, name="nmax", tag="nmax")
            nc.vector.tensor_scalar_mul(nmax[:], rmax[:], -1.0)

            g = gp.tile([P, E], F32, name=f"g{m}", tag=f"g{m}")
            den = small.tile([P, 1], F32, name="den", tag="den")
            nc.scalar.activation(g[:], pg[:], mybir.ActivationFunctionType.Exp,
                                 bias=nmax[:], scale=1.0, accum_out=den[:])
            rden = small.tile([P, 1], F32, name="rden", tag="rden")
            nc.vector.reciprocal(rden[:], den[:])
            nc.vector.tensor_scalar_mul(g[:], g[:], rden[:])
            gates_sb.append(g)

            pt = tps.tile([E, P], F32, name="pt", tag="pt")
            nc.tensor.transpose(pt[:], g[:], ident[:])
            nc.scalar.copy(gT_all[:, ms], pt[:])

        # ---- experts + combine ----
        for n in range(NCH):
            ns = slice(n * NT, (n + 1) * NT)
            accs = []
            for m in range(MC):
                ms = slice(m * P, (m + 1) * P)
                pb = bps.tile([P, NT], F32, name="pb", tag="pb")
                nc.tensor.matmul(pb[:], gT_all[:, ms], be_sb[:, ns],
                                 start=True, stop=True)
                acc = accp.tile([P, NT], F32, name=f"acc{m}", tag=f"acc{m}")
                nc.scalar.copy(acc[:], pb[:])
                accs.append(acc)
            for e in range(E):
                for m in range(MC):
                    ms = slice(m * P, (m + 1) * P)
                    pe = eps.tile([P, NT], F32, name="pe", tag="pe")
                    for k in range(KC):
                        nc.tensor.matmul(pe[:], xt_sb[k][:, ms],
                                         we_sb[e][k][:, ns],
                                         start=(k == 0), stop=(k == KC - 1))
                    nc.vector.scalar_tensor_tensor(
                        accs[m][:], pe[:], gates_sb[m][:, e:e + 1], accs[m][:],
                        MULT, ADD)
            for m in range(MC):
                nc.sync.dma_start(out[m * P:(m + 1) * P, ns], accs[m][:])


_NC_CACHE = {}


def _build():
    if "nc" in _NC_CACHE:
        return _NC_CACHE["nc"]
    nc = bacc.Bacc("TRN2", target_bir_lowering=False, debug=False,
                   num_devices=NCORES)
    xT = nc.dram_tensor("xT", [D, BS], BF16, kind="ExternalInput").ap()
    We_t = nc.dram_tensor("We", [E, D, O], BF16, kind="ExternalInput").ap()
    Wg_t = nc.dram_tensor("Wg", [D, E], BF16, kind="ExternalInput").ap()
    bg_t = nc.dram_tensor("bg", [1, E], F32, kind="ExternalInput").ap()
    be_t = nc.dram_tensor("be", [E, O], BF16, kind="ExternalInput").ap()
    out = nc.dram_tensor("out", [BS, O], F32, kind="ExternalOutput").ap()
    with tile.TileContext(nc) as tc:
        _emit(nc, tc, xT, We_t, Wg_t, bg_t, be_t, out)
    nc.compile()
    _NC_CACHE["nc"] = nc
    return nc


def _in_maps(x, Wg, bg, We, be):
    bf = ml_dtypes.bfloat16
    x = np.asarray(x, dtype=np.float32)
    We_bf = np.asarray(We, dtype=np.float32).astype(bf)
    Wg_bf = np.asarray(Wg, dtype=np.float32).astype(bf)
    be_bf = np.asarray(be, dtype=np.float32).astype(bf)
    bg32 = np.asarray(bg, dtype=np.float32).reshape(1, E)
    maps = []
    for c in range(NCORES):
        xT = np.ascontiguousarray(x[c * BS:(c + 1) * BS].T).astype(bf)
        maps.append({"xT": xT, "We": We_bf, "Wg": Wg_bf,
                     "bg": bg32, "be": be_bf})
    return maps


def run(x, Wg, bg, We, be, **spmd_kwargs):
    nc = _build()
    maps = _in_maps(x, Wg, bg, We, be)
    res = run_bass_kernel_spmd(nc, maps, core_ids=list(range(NCORES)),
                               **spmd_kwargs)
    out = np.concatenate([res.results[c]["out"] for c in range(NCORES)],
                         axis=0)
    return out, res


def kernel(x, Wg, bg, We, be):
    out, _ = run(x, Wg, bg, We, be)
    return out
